# revision 1
# baseline (speedup 1.0000x reference)
"""Trainium2 Bass kernel for fused multi-head attention (dense transformer block).

Problem: y = proj(softmax(QK^T/sqrt(d)) V) for x [4, 2048, 512], 16 heads, d=32.

Sharding (8 cores): core c handles batch b = c//2 and head-group hg = c%2
(8 heads each). Everything on-chip is computed transposed (feature-major)
so softmax denominators and the output projection need no transposes:

  x^T  [c, t]        pre-transposed on the host, loaded via casting DMA
  Q^T/K^T [d, t]     = W^T x^T   (lhsT = W slices, natural layout)
  V    [t, d]        = x W_v     (lhsT = x^T tiles)
  S^T  [k, q]        = K^T.T @ Q^T  per head (contraction d=32, f32r)
  P^T  = exp(S^T / sqrt(d))      (ScalarE, fused scale; scores are small so
                                  no max-subtraction is needed: |s| < ~2)
  [sums | 0 | O^T] = [1 | 0 | V].T @ P^T  per head (M=64 matmul; row 0 =
                                  softmax denominators, rows 32:64 = O^T,
                                  accumulated over k-tiles in one PSUM bank)
  O^T /= sums                    (reciprocal row 0 + partition broadcast + mul)
  out^T [c_out, t]   = W_p.T @ O^T  (lhsT = w_proj rows, natural layout)

V-bias and output bias are folded on the host into a single vector added
after the cross-head-group reduction (softmax rows sum to 1, so the V bias
contributes exactly b_v @ w_proj to every token).

PSUM budget (8 banks): sT tag [128, 2, 512] x 2 bufs = 4 banks; os0..os3
tags [128, 512] x 1 buf = 4 banks (shared by QKV and projection evictions
and the per-head O/sums accumulators — prep and projection run strictly
before/after the attention stage, so there is no slot contention).
"""

import sys

sys.path.insert(0, "/opt/trn_rl_repo")

import numpy as np

N_CORES = 8
B, T, C = 4, 2048, 512
N_HEADS, HEAD_DIM = 16, 32
HPC = 8           # heads per core
NG = 2            # head groups of 4 per core
SCALE = 1.0 / np.sqrt(np.float32(HEAD_DIM))
CT = C // 128     # 4 c-tiles
TT = T // 128     # 16 t-tiles
QC = T // 512     # 4 q-chunks
KT = T // 128     # 16 k-tiles

_CACHE = {}


def _build(stages=("prep", "attn", "proj")):
    import concourse.bass as bass
    import concourse.tile as tile
    from concourse import bacc, mybir

    f32 = mybir.dt.float32
    f32r = mybir.dt.float32r
    Exp = mybir.ActivationFunctionType.Exp
    ts = bass.ts
    ds = bass.ds

    nc = bacc.Bacc("TRN2", target_bir_lowering=False, debug=False,
                   num_devices=N_CORES)

    xT_d = nc.dram_tensor("xT", (CT, 128, T), f32r, kind="ExternalInput")
    wq_d = nc.dram_tensor("wq", (C, 256), f32r, kind="ExternalInput")
    wk_d = nc.dram_tensor("wk", (C, 256), f32r, kind="ExternalInput")
    wv_d = nc.dram_tensor("wv", (C, 256), f32r, kind="ExternalInput")
    bq_d = nc.dram_tensor("bq", (4, 64, 1), f32, kind="ExternalInput")
    bk_d = nc.dram_tensor("bk", (4, 64, 1), f32, kind="ExternalInput")
    wp_d = nc.dram_tensor("wp", (256, C), f32r, kind="ExternalInput")
    outT_d = nc.dram_tensor("outT", (C, T), f32, kind="ExternalOutput")

    from contextlib import ExitStack

    with tile.TileContext(nc) as tc, ExitStack() as ctx:
        persist = ctx.enter_context(tc.tile_pool(name="persist", bufs=1))
        psum = ctx.enter_context(tc.tile_pool(name="psum", bufs=1,
                                              space="PSUM"))
        misc = ctx.enter_context(tc.tile_pool(name="misc", bufs=2))
        pTp = ctx.enter_context(tc.tile_pool(name="pTp", bufs=2))

        def os_tile(i, name):
            return psum.tile([128, 512], f32, tag=f"os{i}", name=name)

        # ---- persistent SBUF ----
        wp = persist.tile([128, 2, C], f32r)
        bq = persist.tile([64, 4], f32)
        bk = persist.tile([64, 4], f32)
        # head-pair layout: [64 partitions (2 heads x 32 d), pair, t]
        # pair p = 2*g + (h // 2); head within pair at partition 32*(h % 2)
        qT = persist.tile([64, 4, T], f32r)
        kT = persist.tile([64, 4, T], f32r)
        # vx: [t-in-tile, t-tile, core-head, 64]:
        #   col 0 = ones, cols 1:32 = zeros, cols 32:64 = V_h
        vx = persist.tile([128, TT, HPC, 64], f32r)
        oT = persist.tile([128, 2, T], f32r)   # [d-in-tile, g, q]
        ones128 = persist.tile([128, 128], f32)
        nc.vector.memset(ones128[:], 1.0)
        nc.vector.tensor_copy(
            vx[:, :, :, 0:1],
            ones128[:].rearrange("p (a b one) -> p a b one", a=TT, b=HPC,
                                 one=1),
        )
        zero31 = persist.tile([128, HPC, 31], f32)
        nc.vector.memset(zero31[:], 0.0)

        def emit_loads(wq, wk, wv):
            for kc in range(CT):
                nc.sync.dma_start(wq[:, kc, :], wq_d.ap()[ts(kc, 128), :])
                nc.sync.dma_start(wk[:, kc, :], wk_d.ap()[ts(kc, 128), :])
                nc.sync.dma_start(wv[:, kc, :], wv_d.ap()[ts(kc, 128), :])
            for yt in range(2):
                nc.sync.dma_start(wp[:, yt, :], wp_d.ap()[ts(yt, 128), :])
            for m in range(4):
                nc.sync.dma_start(bq[:, m:m + 1], bq_d.ap()[m])
                nc.sync.dma_start(bk[:, m:m + 1], bk_d.ap()[m])

        def emit_x_chunk(xT, tch):
            for kc in range(CT):
                nc.sync.dma_start(xT[:, kc, ts(tch, 512)],
                                  xT_d.ap()[kc, :, ts(tch, 512)])

        def emit_qk(xT, wq, wk, g, only=None):
            for tch in (range(QC) if only is None else [only]):
                for pp in range(2):
                    p = 2 * g + pp
                    qps = psum.tile([128, 512], f32, tag="sT", bufs=2,
                                    name="qps")
                    for kc in range(CT):
                        nc.tensor.matmul(
                            qps[0:64, :], wq[:, kc, ds(64 * p, 64)],
                            xT[:, kc, ts(tch, 512)],
                            start=(kc == 0), stop=(kc == CT - 1),
                        )
                    nc.vector.tensor_scalar_add(
                        qT[:, p, ts(tch, 512)], qps[0:64, :],
                        bq[:, p:p + 1])
                    kps = psum.tile([128, 512], f32, tag="sT", bufs=2,
                                    name="kps")
                    for kc in range(CT):
                        nc.tensor.matmul(
                            kps[0:64, :], wk[:, kc, ds(64 * p, 64)],
                            xT[:, kc, ts(tch, 512)],
                            start=(kc == 0), stop=(kc == CT - 1),
                        )
                    nc.vector.tensor_scalar_add(
                        kT[:, p, ts(tch, 512)], kps[0:64, :],
                        bk[:, p:p + 1])

        def emit_v(xT, wv, tch=None):
            tts = range(TT) if tch is None else range(4 * tch, 4 * tch + 4)
            for tt in tts:
                vps = psum.tile([128, 512], f32, tag="sT", bufs=2,
                                name="vps")
                for kc in range(CT):
                    nc.tensor.matmul(
                        vps[:, 0:256], xT[:, kc, ts(tt, 128)], wv[:, kc, :],
                        start=(kc == 0), stop=(kc == CT - 1),
                    )
                nc.vector.tensor_copy(
                    vx[:, tt, :, 32:64],
                    vps[:, 0:256].rearrange("p (h d) -> p h d", h=HPC),
                )
                nc.vector.tensor_copy(vx[:, tt, :, 1:32], zero31[:])

        def emit_attn_kts(g, qc, os_ps, kts):
            for kt in kts:
                pT = pTp.tile([128, 4, 512], f32r, tag="pT", name="pT")
                # two half-groups (= head pairs) so exp(half A) overlaps
                # the QK^T matmuls of half B on the PE
                for half in range(2):
                    p = 2 * g + half
                    sT = psum.tile([128, 2, 512], f32, tag="sT", bufs=2,
                                   name="sT")
                    for i in range(2):
                        nc.tensor.matmul(
                            sT[:, i, :],
                            kT[ds(32 * i, 32), p, ts(kt, 128)],
                            qT[ds(32 * i, 32), p, ts(qc, 512)],
                            start=True, stop=True,
                            tile_position=(32 * i, 0),
                        )
                    nc.scalar.activation(
                        pT[:, 2 * half:2 * half + 2, :], sT[:],
                        Exp, scale=float(SCALE))
                for h in range(4):
                    nc.tensor.matmul(
                        os_ps[h][0:64, :],
                        vx[:, kt, 4 * g + h, :],
                        pT[:, h, :],
                        start=(kt == 0), stop=(kt == KT - 1),
                    )

        def emit_norm(g, qc, os_ps):
                for h in range(4):
                    r_sb = misc.tile([1, 512], f32, tag="r_sb", name="r_sb")
                    nc.vector.reciprocal(r_sb[:], os_ps[h][0:1, :])
                    rr = misc.tile([64, 512], f32, tag="rr", name="rr")
                    nc.gpsimd.partition_broadcast(rr[:], r_sb[:])
                    ot = misc.tile([64, 512], f32r, tag="ot", name="ot")
                    nc.vector.tensor_mul(ot[32:64, :], os_ps[h][32:64, :],
                                         rr[32:64, :])
                    nc.sync.dma_start(oT[ts(h, 32), g, ts(qc, 512)],
                                      ot[32:64, :])

        def emit_attention(g, skip_qc0=False):
            for qc in range(1 if skip_qc0 else 0, QC):
                os_ps = [os_tile(h, f"os{h}") for h in range(4)]
                emit_attn_kts(g, qc, os_ps, range(KT))
                emit_norm(g, qc, os_ps)

        def emit_proj(ostage, tch):
            for ct in range(CT):
                pps = os_tile(ct, "pps")
                for yt in range(2):
                    nc.tensor.matmul(
                        pps[:], wp[:, yt, ts(ct, 128)],
                        oT[:, yt, ts(tch, 512)],
                        start=(yt == 0), stop=(yt == 1),
                    )
                ost = ostage.tile([128, 512], f32, tag="ost", name="ost")
                nc.vector.tensor_copy(ost[:], pps[:])
                nc.sync.dma_start(
                    outT_d.ap()[ts(ct, 128), ts(tch, 512)], ost[:])

        with tc.tile_pool(name="xTp", bufs=1) as xTp:
            wq = xTp.tile([128, CT, 256], f32r)
            wk = xTp.tile([128, CT, 256], f32r)
            wv = xTp.tile([128, CT, 256], f32r)
            xT = xTp.tile([128, CT, T], f32r)
            emit_loads(wq, wk, wv)
            for tch in range(QC):
                emit_x_chunk(xT, tch)
            os_ps0 = None
            for tch in range(QC):
                emit_qk(xT, wq, wk, 0, only=tch)
                emit_v(xT, wv, tch=tch)
                if "attn" in stages:
                    if tch == 0:
                        os_ps0 = [os_tile(h, f"os{h}") for h in range(4)]
                    emit_attn_kts(0, 0, os_ps0,
                                  range(4 * tch, 4 * tch + 4))
                emit_qk(xT, wq, wk, 1, only=tch)
            if "attn" in stages:
                emit_norm(0, 0, os_ps0)
                emit_attention(0, skip_qc0=True)
        if "attn" in stages:
            emit_attention(1)
        if "proj" in stages:
            with tc.tile_pool(name="ostage", bufs=6) as ostage:
                for tch in range(QC):
                    emit_proj(ostage, tch)

    nc.compile()
    return nc


def _get_nc():
    if "nc" not in _CACHE:
        _CACHE["nc"] = _build()
    return _CACHE["nc"]


def kernel(x, w_attn, b_attn, w_proj, b_proj):
    from concourse.bass_utils import run_bass_kernel_spmd

    x = np.asarray(x, dtype=np.float32)
    w_attn = np.asarray(w_attn, dtype=np.float32)
    b_attn = np.asarray(b_attn, dtype=np.float32)
    w_proj = np.ascontiguousarray(np.asarray(w_proj, dtype=np.float32))
    b_proj = np.asarray(b_proj, dtype=np.float32)

    nc = _get_nc()

    in_maps = []
    for core in range(N_CORES):
        b, hg = core // 2, core % 2
        cs = hg * 256  # head-column offset within each of q/k/v blocks
        wq = np.ascontiguousarray(w_attn[:, cs:cs + 256])
        wk = np.ascontiguousarray(w_attn[:, C + cs:C + cs + 256])
        wv = np.ascontiguousarray(w_attn[:, 2 * C + cs:2 * C + cs + 256])
        bq = np.ascontiguousarray(b_attn[cs:cs + 256].reshape(4, 64, 1))
        bk = np.ascontiguousarray(
            b_attn[C + cs:C + cs + 256].reshape(4, 64, 1))
        wp = np.ascontiguousarray(w_proj[cs:cs + 256, :])
        xT = np.ascontiguousarray(x[b].T).reshape(CT, 128, T)
        in_maps.append({
            "xT": xT,
            "wq": wq, "wk": wk, "wv": wv, "bq": bq, "bk": bk, "wp": wp,
        })

    res = run_bass_kernel_spmd(nc, in_maps, core_ids=list(range(N_CORES)))

    b_eff = (b_proj + b_attn[2 * C:3 * C] @ w_proj).astype(np.float32)
    out = np.empty((B, T, C), dtype=np.float32)
    for b in range(B):
        acc = res.results[2 * b]["outT"].T + res.results[2 * b + 1]["outT"].T
        out[b] = acc + b_eff
    return out


if __name__ == "__main__":
    rng = np.random.default_rng(0)
    x = rng.standard_normal((B, T, C), dtype=np.float32)
    w_attn = (rng.standard_normal((C, 3 * C), dtype=np.float32) * 0.02)
    b_attn = (rng.standard_normal(3 * C, dtype=np.float32) * 0.02)
    w_proj = (rng.standard_normal((C, C), dtype=np.float32) * 0.02)
    b_proj = (rng.standard_normal(C, dtype=np.float32) * 0.02)
    out = kernel(x, w_attn, b_attn, w_proj, b_proj)
    print("kernel out", out.shape, out.dtype, float(np.abs(out).max()))



# revision 15
# speedup vs baseline: 1.6492x; 1.6492x over previous
"""Trainium2 Bass kernel for fused multi-head attention (dense transformer).

y = proj(softmax(QK^T/sqrt(d)) V) for x [4, 2048, 512], 16 heads, d=32.

Sharding (8 cores): core c handles batch b = c//2 and head-group hg = c%2
(8 heads each); the two half-results per batch are summed on the host.

Everything on-chip is feature-major (transposed) so softmax denominators
and the projection need no transposes. All matmuls run in fp8e4m3 with
DoubleRow perf mode (0.5 PE cycles/row, K=256 per instruction); weights
are pre-scaled by 64 on the host (avoids fp8 subnormals; the exact 1/4096
is folded into the exp scale and the output eviction).

  x8   [128,2a,2i,T] fp8    c = 128*(2a+i)+p  (DR-packed contraction)
  q/k  = W^T x  -> psum [64(2 heads x 32d), 512] -> +bias -> qT8/kT8
         [64, 4hp, 2i, T] fp8 (i=1 of kT8 zeroed: DR zero-pad trick)
  V    = x W_v -> vx8 [128, 8ktp, 2i, 8h, 64] fp8, col0=1 (denominator
         trick), cols 1:32 zero, cols 32:64 = V_h
  S^T  pair tile [128, 2kt, 512] psum per (head, ktpair): 1 DR matmul per
         kt (lhsT = kT8 32-partition slice, zero-padded i)
  P^T  = exp(S^T * scale/4096): split across ACT (native exp -> fp8) and
         DVE/Pool (1-op Schraudolph: int8 bitcast of fp8, A*s+B -> rint)
  [sums|0|O^T] += vx8^T P^T  per (head, ktpair): 1 DR matmul into os[j]
  O^T /= sums (DVE reciprocal row -> Pool partition_broadcast -> mul),
         written fp8 and SBUF-DMA'd into oT8 [128, 2j, T]
  out^T = wp8^T oT8 (DR, K=256) * (1/4096) -> outT [512, T] f32

A greedy per-engine load balancer assigns each flexible op (exp tiles,
evictions, norm muls) to the least-busy engine (ACT/DVE/Pool) using the
cost-model rates.

PSUM (8 banks): sT tag [128,2,512] x 2 bufs = 4 banks; os0..os3
[128,512] = 4 banks (also reused for V-prep and projection psum).
"""

import os
import sys

sys.path.insert(0, "/opt/trn_rl_repo")

import numpy as np
import ml_dtypes

F8 = ml_dtypes.float8_e4m3
BF16 = ml_dtypes.bfloat16

N_CORES = 8
B, T, C = 4, 2048, 512
N_HEADS, HEAD_DIM = 16, 32
HPC = 8            # heads per core
SCALE = 1.0 / np.sqrt(np.float32(HEAD_DIM))
W_SCALE = 64.0     # host-side weight prescale (exact power of 2)
QK_FACT = W_SCALE * W_SCALE  # 4096: scores come out multiplied by this
LN2 = float(np.log(2.0))
# Schraudolph-in-fp8: bits = rint(s * A8 + B8); bitcast int8 -> fp8e4m3
# optimal 2.0403 per unscaled-score unit; psum holds 4096*s
A8 = 2.0403 / 4096.0
B8 = 55.625

S_FP8 = os.environ.get("S_FP8", "1") == "1"

QC = 4             # q chunks of 512
KT = 16            # k tiles of 128
KTP = 8            # k tile pairs
TT = 16            # t tiles of 128 (for V)

_CACHE = {}


def _build(s_fp8=S_FP8):
    import concourse.bass as bass
    import concourse.tile as tile
    from concourse import bacc, mybir

    f32 = mybir.dt.float32
    f32r = mybir.dt.float32r
    f8 = mybir.dt.float8e4
    i8 = mybir.dt.int8
    bf16 = mybir.dt.bfloat16
    Exp = mybir.ActivationFunctionType.Exp
    Identity = mybir.ActivationFunctionType.Identity
    Copy = mybir.ActivationFunctionType.Copy
    DR = mybir.MatmulPerfMode.DoubleRow
    MUL = mybir.AluOpType.mult
    ADD = mybir.AluOpType.add
    ts = bass.ts
    ds = bass.ds

    nc = bacc.Bacc("TRN2", target_bir_lowering=False, debug=False,
                   num_devices=N_CORES)

    x8_d = nc.dram_tensor("xb", (128, 4, T), bf16, kind="ExternalInput")
    wq_d = nc.dram_tensor("wqb", (128, 4, 256), bf16, kind="ExternalInput")
    wk_d = nc.dram_tensor("wkb", (128, 4, 256), bf16, kind="ExternalInput")
    wv_d = nc.dram_tensor("wvb", (128, 4, 256), bf16, kind="ExternalInput")
    bq_d = nc.dram_tensor("bq", (128, 2), f32, kind="ExternalInput")
    bk_d = nc.dram_tensor("bk", (128, 2), f32, kind="ExternalInput")
    wp_d = nc.dram_tensor("wpb", (128, 2, C), bf16, kind="ExternalInput")
    outT_d = nc.dram_tensor("outT", (C, T), f32, kind="ExternalOutput")

    from contextlib import ExitStack

    # greedy engine balancer (estimated busy ns per engine)
    bal = {"act": 0.0, "dve": 0.0, "pool": 0.0}

    def pick(opts):
        e = min(opts, key=lambda ec: bal[ec[0]] + ec[1])
        bal[e[0]] += e[1]
        return e[0]

    def c_act(free):
        return 0.8333 * free + 195.0

    def c_dve(free):
        return 1.0417 * free + 150.0

    def c_pool(free, eff=0.6):
        return 0.8333 / eff * free + 105.0

    # per-head AV output placement: O^T at partitions [c0, c0+32),
    # softmax sums at partition sp (= the ones-column index)
    def c0_of(h):
        return 32 * (h % 4)

    def sp_of(h):
        return 32 if h % 4 == 0 else 0

    with tile.TileContext(nc) as tc, ExitStack() as ctx:
        persist = ctx.enter_context(tc.tile_pool(name="persist", bufs=1))
        psum = ctx.enter_context(tc.tile_pool(name="psum", bufs=1,
                                              space="PSUM"))
        misc = ctx.enter_context(tc.tile_pool(name="misc", bufs=3))
        pTp = ctx.enter_context(tc.tile_pool(name="pTp", bufs=6))
        ostage = ctx.enter_context(tc.tile_pool(name="ostage", bufs=4))

        # ---- persistent SBUF ----
        x8 = persist.tile([128, 4, T], bf16)
        wq8 = persist.tile([128, 4, 256], bf16)
        wk8 = persist.tile([128, 4, 256], bf16)
        wv8 = persist.tile([128, 4, 256], bf16)
        wp8 = persist.tile([128, 2, C], bf16)
        bq = persist.tile([128, 2], f32)
        bk = persist.tile([128, 2], f32)
        if s_fp8:
            qT = persist.tile([64, 4, 2, T], f8)
            kT = persist.tile([64, 4, 2, T], f8)
        else:
            qT = persist.tile([64, 4, T], f32r)
            kT = persist.tile([64, 4, T], f32r)
        # vx8[p, ktp, i, u, m, c]: padded m-stride-160 layout. Head h=4u+m
        # reads lhsT cols [0:128) at flat offset 128m within the u-block:
        # V_h lands at cols [32m, 32m+32) and the 1.0 denominator column at
        # col sp(h); other cols read neighbouring data (their psum rows are
        # never consumed).
        vx8 = persist.tile([128, KTP, 2, 2, 4, 160], f8)
        oT8 = persist.tile([128, 2, T], bf16)

        # ---- loads (x8 chunk 0 + qk weights + biases first) ----
        nc.sync.dma_start(wq8[:], wq_d.ap())
        nc.sync.dma_start(wk8[:], wk_d.ap())
        nc.sync.dma_start(x8[:, :, ts(0, 512)],
                          x8_d.ap()[:, :, ts(0, 512)])
        nc.sync.dma_start(bq[:], bq_d.ap())
        nc.sync.dma_start(bk[:], bk_d.ap())
        nc.sync.dma_start(wv8[:], wv_d.ap())
        for tch in range(1, QC):
            nc.sync.dma_start(x8[:, :, ts(tch, 512)],
                              x8_d.ap()[:, :, ts(tch, 512)])
        nc.sync.dma_start(wp8[:], wp_d.ap())

        # ---- constant regions ----
        # ones column for head h=4u+m sits at u-block offset 128m + sp(h),
        # i.e. tile coords (m', c') with 160*m' + c' = 128m + sp
        for m, (mp, cp) in enumerate(((0, 32), (0, 128), (1, 96), (2, 64))):
            nc.gpsimd.memset(vx8[:, :, :, :, mp, cp:cp + 1], 1.0)
            bal["pool"] += c_pool(32, eff=1.0)
        if s_fp8:
            nc.gpsimd.memset(kT[:, :, 1, :], 0.0)
            bal["pool"] += c_pool(8192, eff=1.0)
            # qT i=1 must be finite too: fp8 garbage can be NaN and the
            # DoubleRow i=1 term would produce 0*NaN = NaN
            nc.gpsimd.memset(qT[:, :, 1, :], 0.0)
            bal["pool"] += c_pool(8192, eff=1.0)

        def sT_tile():
            return psum.tile([128, 2, 512], f32, tag="sT", bufs=3,
                             name="sT")

        def os_tile(i, name):
            return psum.tile([128, 512], f32, tag=f"os{i}", name=name)

        def emit_evict_qk(dst_ap, src_ap, bias_ap):
            e = pick([("act", c_act(512)), ("dve", c_dve(512))])
            if e == "act":
                nc.scalar.activation(dst_ap, src_ap, Identity, bias=bias_ap)
            else:
                nc.vector.tensor_scalar_add(dst_ap, src_ap, bias_ap)

        def emit_copy(dst_ap, src_ap, free, scale=None):
            e = pick([("act", c_act(free)), ("dve", c_dve(free))])
            if e == "act":
                if scale is None:
                    nc.scalar.activation(dst_ap, src_ap, Copy)
                else:
                    nc.scalar.activation(dst_ap, src_ap, Copy, scale=scale)
            else:
                if scale is None:
                    nc.vector.tensor_copy(dst_ap, src_ap)
                else:
                    nc.vector.tensor_scalar_mul(dst_ap, src_ap, scale)

        def emit_exp(pT_ap, pT_i8_ap, sT_ap):
            e = pick([("act", c_act(1024)), ("dve", c_dve(1024))])
            if e == "act":
                nc.scalar.activation(pT_ap, sT_ap, Exp,
                                     scale=float(SCALE) / QK_FACT)
            else:
                nc.vector.tensor_scalar(pT_i8_ap, sT_ap, A8, B8, MUL, ADD)

        # ---- prep: QKV projections (M=128: 4 heads per matmul) ----
        def emit_qk_tch(tch):
            for g in range(2):
                pt = sT_tile()
                for qk, w8 in enumerate((wq8, wk8)):
                    for kc in range(4):
                        nc.tensor.matmul(
                            pt[:, qk, :],
                            w8[:, kc, ds(128 * g, 128)],
                            x8[:, kc, ts(tch, 512)],
                            start=(kc == 0), stop=(kc == 3),
                        )
                for qk, (bt, dT) in enumerate(((bq, qT), (bk, kT))):
                    for half in range(2):   # heads 4g+{0,1} then 4g+{2,3}
                        hp = 2 * g + half
                        if s_fp8:
                            dst = dT[0:64, hp, 0, ts(tch, 512)]
                        else:
                            dst = dT[0:64, hp, ts(tch, 512)]
                        emit_evict_qk(dst, pt[ds(64 * half, 64), qk, :],
                                      bt[ds(64 * half, 64), g:g + 1])

        def emit_v_tch(tch):
            for tt in range(4 * tch, 4 * tch + 4):
                vt = os_tile(tt % 2, "vps")
                for kc in range(4):
                    nc.tensor.matmul(
                        vt[:, 0:256],
                        x8[:, kc, ts(tt, 128)],
                        wv8[:, kc, :],
                        start=(kc == 0), stop=(kc == 3),
                    )
                dst = vx8[:, tt // 2, tt % 2, :, :, 0:32]
                e = pick([("act", c_act(256)), ("dve", c_dve(256))])
                src = vt[:, 0:256].rearrange("p (u m d) -> p u m d",
                                             u=2, m=4)
                if e == "act":
                    nc.scalar.activation(dst, src, Copy)
                else:
                    nc.vector.tensor_copy(dst, src)

        emit_qk_tch(0)
        emit_v_tch(0)
        emit_v_tch(1)
        for tch in range(1, QC):
            emit_qk_tch(tch)
        emit_v_tch(2)
        emit_v_tch(3)

        # ---- attention (head-serial) ----
        def emit_head(qc, h, os_ps, deferred=(), eager=False):
            hp, a = h // 2, h % 2
            dq = list(deferred)
            for ktp in range(KTP):
                if dq and (eager or ktp in (2, 5)):
                    dq.pop(0)()
                st = sT_tile()
                for i in range(2):
                    kt = 2 * ktp + i
                    if s_fp8:
                        nc.tensor.matmul(
                            st[:, i, :],
                            kT[ds(32 * a, 32), hp, :, ts(kt, 128)],
                            qT[ds(32 * a, 32), hp, :, ts(qc, 512)],
                            start=True, stop=True, perf_mode=DR,
                        )
                    else:
                        nc.tensor.matmul(
                            st[:, i, :],
                            kT[ds(32 * a, 32), hp, ts(kt, 128)],
                            qT[ds(32 * a, 32), hp, ts(qc, 512)],
                            start=True, stop=True,
                            tile_position=(32 * a, 0),
                        )
                pT = pTp.tile([128, 2, 512], f8, tag="pT", name="pT")
                emit_exp(pT[:], pT[:].bitcast(i8), st[:])
                u, m = h // 4, h % 4
                lhsT = vx8[:, ktp, :, u, :, :].rearrange(
                    "p i m c -> p i (m c)")[:, :, 128 * m:128 * m + 128]
                nc.tensor.matmul(
                    os_ps[0:128, :],
                    lhsT,
                    pT[:],
                    start=(ktp == 0), stop=(ktp == KTP - 1),
                    perf_mode=DR,
                )

        def emit_norm(qc, h, os_ps):
            c0, sp = c0_of(h), sp_of(h)
            r_sb = misc.tile([1, 512], f32, tag="r_sb", name="r_sb")
            nc.vector.reciprocal(r_sb[0:1, :], os_ps[ds(sp, 1), :])
            bal["dve"] += c_dve(512)
            rr = misc.tile([128, 512], f32, tag="rr", name="rr")
            nc.gpsimd.partition_broadcast(rr[:], r_sb[:])
            bal["pool"] += c_pool(512)
            dst = oT8[ds(c0, 32), h // 4, ts(qc, 512)]
            # path A: one DVE mul from psum; path B: ACT evict + Pool mul
            pool_mul = c_pool(512, eff=0.42)
            if bal["dve"] + c_dve(512) <= max(bal["act"] + c_act(512),
                                              bal["pool"] + pool_mul):
                e = "dve"
                bal["dve"] += c_dve(512)
            else:
                e = "actpool"
                bal["act"] += c_act(512)
            if e == "dve":
                nc.vector.tensor_mul(dst, os_ps[ds(c0, 32), :],
                                     rr[ds(c0, 32), :])
            else:
                o32 = misc.tile([128, 512], f32, tag="o32", name="o32")
                nc.scalar.activation(o32[ds(c0, 32), :],
                                     os_ps[ds(c0, 32), :], Copy)
                nc.gpsimd.tensor_mul(dst, o32[ds(c0, 32), :],
                                     rr[ds(c0, 32), :])
                bal["pool"] += c_pool(512, eff=0.42)

        def emit_proj_ct(qc, ct):
            pps = sT_tile()
            for yt in range(2):
                nc.tensor.matmul(
                    pps[:, 0, :], wp8[:, yt, ts(ct, 128)],
                    oT8[:, yt, ts(qc, 512)],
                    start=(yt == 0), stop=(yt == 1),
                )
            ost = ostage.tile([128, 512], f32, tag="ost", name="ost")
            emit_copy(ost[:], pps[:, 0, :], 512, scale=1.0 / W_SCALE)
            nc.sync.dma_start(
                outT_d.ap()[ts(ct, 128), ts(qc, 512)], ost[:])

        from collections import deque
        pending = deque()

        def make_norm(qc, h, os_ps):
            def f():
                emit_norm(qc, h, os_ps)
            return f

        def make_proj(qc, ct):
            def f():
                emit_proj_ct(qc, ct)
            return f

        for qc in range(QC):
            for h in range(HPC):
                os_ps = os_tile(h % 2, f"os{h % 2}")
                eager = qc == QC - 1 and h >= 6
                todo = []
                for _ in range(8 if eager else 2):
                    if pending:
                        todo.append(pending.popleft())
                emit_head(qc, h, os_ps, deferred=todo, eager=eager)
                pending.append(make_norm(qc, h, os_ps))
            # last head norm of this qc must land before its proj
            if qc < QC - 1:
                for ct in range(4):
                    pending.append(make_proj(qc, ct))
        while pending:
            pending.popleft()()
        for ct in range(4):
            emit_proj_ct(QC - 1, ct)

    nc.compile()
    nc._engine_balance = dict(bal)
    return nc


def _get_nc():
    if "nc" not in _CACHE:
        _CACHE["nc"] = _build()
    return _CACHE["nc"]


def kernel(x, w_attn, b_attn, w_proj, b_proj):
    from concourse.bass_utils import run_bass_kernel_spmd

    x = np.asarray(x, dtype=np.float32)
    w_attn = np.asarray(w_attn, dtype=np.float32)
    b_attn = np.asarray(b_attn, dtype=np.float32)
    w_proj = np.asarray(w_proj, dtype=np.float32)
    b_proj = np.asarray(b_proj, dtype=np.float32)

    nc = _get_nc()

    in_maps = []
    for core in range(N_CORES):
        b, hg = core // 2, core % 2
        cs = hg * 256
        # xb[p, kc, t] = x[b, t, 128*kc+p]
        x8 = np.ascontiguousarray(
            x[b].T.reshape(4, 128, T).transpose(1, 0, 2)).astype(BF16)

        def wpack(w):  # [C, 256] -> [128, 4, 256], scaled, bf16
            return np.ascontiguousarray(
                (w * W_SCALE).reshape(4, 128, 256)
                .transpose(1, 0, 2)).astype(BF16)

        wq8 = wpack(w_attn[:, cs:cs + 256])
        wk8 = wpack(w_attn[:, C + cs:C + cs + 256])
        wv8 = wpack(w_attn[:, 2 * C + cs:2 * C + cs + 256])
        bq = np.ascontiguousarray(
            (b_attn[cs:cs + 256] * W_SCALE).reshape(2, 128).T)
        bk = np.ascontiguousarray(
            (b_attn[C + cs:C + cs + 256] * W_SCALE).reshape(2, 128).T)
        # wp8[p, j, n] = w_proj[cs + 128j + p, n] * 64
        wp8 = np.ascontiguousarray(
            w_proj[cs:cs + 256, :].reshape(2, 128, C)
            .transpose(1, 0, 2)).astype(BF16)
        in_maps.append({
            "xb": x8, "wqb": wq8, "wkb": wk8, "wvb": wv8,
            "bq": bq.astype(np.float32), "bk": bk.astype(np.float32),
            "wpb": wp8,
        })

    res = run_bass_kernel_spmd(nc, in_maps, core_ids=list(range(N_CORES)))

    b_eff = (b_proj + b_attn[2 * C:3 * C] @ w_proj).astype(np.float32)
    out = np.empty((B, T, C), dtype=np.float32)
    for b in range(B):
        acc = res.results[2 * b]["outT"].T + res.results[2 * b + 1]["outT"].T
        out[b] = acc + b_eff
    return out


if __name__ == "__main__":
    rng = np.random.default_rng(0)
    x = rng.standard_normal((B, T, C), dtype=np.float32)
    w_attn = rng.standard_normal((C, 3 * C), dtype=np.float32) * 0.02
    b_attn = rng.standard_normal(3 * C, dtype=np.float32) * 0.02
    w_proj = rng.standard_normal((C, C), dtype=np.float32) * 0.02
    b_proj = rng.standard_normal(C, dtype=np.float32) * 0.02
    out = kernel(x, w_attn, b_attn, w_proj, b_proj)
    print("kernel out", out.shape, out.dtype, float(np.abs(out).max()))


# revision 23
# speedup vs baseline: 1.7416x; 1.0560x over previous
"""Trainium2 Bass kernel for fused multi-head attention (dense transformer).

y = proj(softmax(QK^T/sqrt(d)) V) for x [4, 2048, 512], 16 heads, d=32.

Sharding (8 cores): core c handles batch b = c//2 and head-group hg = c%2
(8 heads each); the two half-results per batch are summed on the host.

Everything on-chip is feature-major (transposed) so softmax denominators
and the projection need no transposes. All matmuls run in fp8e4m3 with
DoubleRow perf mode (0.5 PE cycles/row, K=256 per instruction); weights
are pre-scaled by 64 on the host (avoids fp8 subnormals; the exact 1/4096
is folded into the exp scale and the output eviction).

  x8   [128,2a,2i,T] fp8    c = 128*(2a+i)+p  (DR-packed contraction)
  q/k  = W^T x  -> psum [64(2 heads x 32d), 512] -> +bias -> qT8/kT8
         [64, 4hp, 2i, T] fp8 (i=1 of kT8 zeroed: DR zero-pad trick)
  V    = x W_v -> vx8 [128, 8ktp, 2i, 8h, 64] fp8, col0=1 (denominator
         trick), cols 1:32 zero, cols 32:64 = V_h
  S^T  pair tile [128, 2kt, 512] psum per (head, ktpair): 1 DR matmul per
         kt (lhsT = kT8 32-partition slice, zero-padded i)
  P^T  = exp(S^T * scale/4096): split across ACT (native exp -> fp8) and
         DVE/Pool (1-op Schraudolph: int8 bitcast of fp8, A*s+B -> rint)
  [sums|0|O^T] += vx8^T P^T  per (head, ktpair): 1 DR matmul into os[j]
  O^T /= sums (DVE reciprocal row -> Pool partition_broadcast -> mul),
         written fp8 and SBUF-DMA'd into oT8 [128, 2j, T]
  out^T = wp8^T oT8 (DR, K=256) * (1/4096) -> outT [512, T] f32

A greedy per-engine load balancer assigns each flexible op (exp tiles,
evictions, norm muls) to the least-busy engine (ACT/DVE/Pool) using the
cost-model rates.

PSUM (8 banks): sT tag [128,2,512] x 2 bufs = 4 banks; os0..os3
[128,512] = 4 banks (also reused for V-prep and projection psum).
"""

import os
import sys

sys.path.insert(0, "/opt/trn_rl_repo")

import numpy as np
import ml_dtypes

F8 = ml_dtypes.float8_e4m3
BF16 = ml_dtypes.bfloat16

N_CORES = 8
B, T, C = 4, 2048, 512
N_HEADS, HEAD_DIM = 16, 32
HPC = 8            # heads per core
SCALE = 1.0 / np.sqrt(np.float32(HEAD_DIM))
W_SCALE = 64.0     # host-side weight prescale (exact power of 2)
QK_FACT = W_SCALE * W_SCALE  # 4096: scores come out multiplied by this
LN2 = float(np.log(2.0))
# Schraudolph-in-fp8: bits = rint(s * A8 + B8); bitcast int8 -> fp8e4m3
# optimal 2.0403 per unscaled-score unit; psum holds 4096*s
A8 = 2.0403 / 4096.0
B8 = 55.625

S_FP8 = os.environ.get("S_FP8", "1") == "1"

QC = 4             # q chunks of 512
KT = 16            # k tiles of 128
KTP = 8            # k tile pairs
TT = 16            # t tiles of 128 (for V)

_CACHE = {}


def _build(s_fp8=S_FP8):
    import concourse.bass as bass
    import concourse.tile as tile
    from concourse import bacc, mybir

    f32 = mybir.dt.float32
    f32r = mybir.dt.float32r
    f8 = mybir.dt.float8e4
    i8 = mybir.dt.int8
    bf16 = mybir.dt.bfloat16
    Exp = mybir.ActivationFunctionType.Exp
    Identity = mybir.ActivationFunctionType.Identity
    Copy = mybir.ActivationFunctionType.Copy
    DR = mybir.MatmulPerfMode.DoubleRow
    MUL = mybir.AluOpType.mult
    ADD = mybir.AluOpType.add
    ts = bass.ts
    ds = bass.ds

    nc = bacc.Bacc("TRN2", target_bir_lowering=False, debug=False,
                   num_devices=N_CORES)

    x8_d = nc.dram_tensor("xb", (128, 4, T), bf16, kind="ExternalInput")
    xf_d = nc.dram_tensor("xf", (128, 2, 2, T), f8, kind="ExternalInput")
    wq_d = nc.dram_tensor("wqf", (128, 2, 2, 256), f8, kind="ExternalInput")
    wk_d = nc.dram_tensor("wkf", (128, 2, 2, 256), f8, kind="ExternalInput")
    wv_d = nc.dram_tensor("wvb", (128, 4, 256), bf16, kind="ExternalInput")
    bq_d = nc.dram_tensor("bq", (128, 2), f32, kind="ExternalInput")
    bk_d = nc.dram_tensor("bk", (128, 2), f32, kind="ExternalInput")
    wp_d = nc.dram_tensor("wpb", (128, 2, C), bf16, kind="ExternalInput")
    outT_d = nc.dram_tensor("outT", (C, T), f32, kind="ExternalOutput")

    from contextlib import ExitStack

    # greedy engine balancer (estimated busy ns per engine)
    bal = {"act": 0.0, "dve": 0.0, "pool": 0.0}

    def pick(opts):
        e = min(opts, key=lambda ec: bal[ec[0]] + ec[1])
        bal[e[0]] += e[1]
        return e[0]

    def c_act(free):
        return 0.8733 * free + 195.0

    def c_dve(free):
        return 1.0017 * free + 150.0

    def c_pool(free, eff=0.6):
        return 0.8333 / eff * free + 105.0

    # per-head AV output placement: O^T at partitions [c0, c0+32),
    # softmax sums at partition sp (= the ones-column index)
    def c0_of(h):
        return 32 * (h % 4)

    def sp_of(h):
        return 32 if h % 4 == 0 else 0

    with tile.TileContext(nc) as tc, ExitStack() as ctx:
        persist = ctx.enter_context(tc.tile_pool(name="persist", bufs=1))
        psum = ctx.enter_context(tc.tile_pool(name="psum", bufs=1,
                                              space="PSUM"))
        misc = ctx.enter_context(tc.tile_pool(name="misc", bufs=3))
        pTp = ctx.enter_context(tc.tile_pool(name="pTp", bufs=6))
        ostage = ctx.enter_context(tc.tile_pool(name="ostage", bufs=4))

        # ---- persistent SBUF ----
        x8 = persist.tile([128, 4, T], bf16)
        xf = persist.tile([128, 2, 2, T], f8)
        wq8 = persist.tile([128, 2, 2, 256], f8)
        wk8 = persist.tile([128, 2, 2, 256], f8)
        wv8 = persist.tile([128, 4, 256], bf16)
        wp8 = persist.tile([128, 2, C], bf16)
        bq = persist.tile([128, 2], f32)
        bk = persist.tile([128, 2], f32)
        if s_fp8:
            qT = persist.tile([64, 4, 2, T], f8)
            kT = persist.tile([64, 4, 2, T], f8)
        else:
            qT = persist.tile([64, 4, T], f32r)
            kT = persist.tile([64, 4, T], f32r)
        # vx8[p, ktp, i, u, m, c]: padded m-stride-160 layout. Head h=4u+m
        # reads lhsT cols [0:128) at flat offset 128m within the u-block:
        # V_h lands at cols [32m, 32m+32) and the 1.0 denominator column at
        # col sp(h); other cols read neighbouring data (their psum rows are
        # never consumed).
        vx8 = persist.tile([128, KTP, 2, 2, 4, 160], f8)
        oT8 = persist.tile([128, 2, T], bf16)

        # ---- loads (x8 chunk 0 + qk weights + biases first) ----
        nc.sync.dma_start(wq8[:], wq_d.ap())
        nc.sync.dma_start(wk8[:], wk_d.ap())
        nc.sync.dma_start(xf[:, :, :, ts(0, 512)],
                          xf_d.ap()[:, :, :, ts(0, 512)])
        nc.sync.dma_start(bq[:], bq_d.ap())
        nc.sync.dma_start(bk[:], bk_d.ap())
        nc.sync.dma_start(wv8[:], wv_d.ap())
        for tch in range(1, QC):
            nc.sync.dma_start(xf[:, :, :, ts(tch, 512)],
                              xf_d.ap()[:, :, :, ts(tch, 512)])
        for tch in range(QC):
            nc.sync.dma_start(x8[:, :, ts(tch, 512)],
                              x8_d.ap()[:, :, ts(tch, 512)])
        nc.sync.dma_start(wp8[:], wp_d.ap())

        # ---- constant regions ----
        # ones column for head h=4u+m sits at u-block offset 128m + sp(h),
        # i.e. tile coords (m', c') with 160*m' + c' = 128m + sp
        for m, (mp, cp) in enumerate(((0, 32), (0, 128), (1, 96), (2, 64))):
            nc.gpsimd.memset(vx8[:, :, :, :, mp, cp:cp + 1], 1.0)
            bal["pool"] += c_pool(32, eff=1.0)
        if s_fp8:
            nc.gpsimd.memset(kT[:, :, 1, :], 0.0)
            bal["pool"] += c_pool(8192, eff=1.0)
            # qT i=1 must be finite too: fp8 garbage can be NaN and the
            # DoubleRow i=1 term would produce 0*NaN = NaN
            nc.gpsimd.memset(qT[:, :, 1, :], 0.0)
            bal["pool"] += c_pool(8192, eff=1.0)

        def sT_tile():
            return psum.tile([128, 2, 512], f32, tag="sT", bufs=3,
                             name="sT")

        def os_tile(i, name):
            return psum.tile([128, 512], f32, tag=f"os{i}", name=name)

        def emit_evict_qk(dst_ap, src_ap, bias_ap):
            e = pick([("act", c_act(512)), ("dve", c_dve(512))])
            if e == "act":
                nc.scalar.activation(dst_ap, src_ap, Identity, bias=bias_ap)
            else:
                nc.vector.tensor_scalar_add(dst_ap, src_ap, bias_ap)

        def emit_copy(dst_ap, src_ap, free, scale=None):
            e = pick([("act", c_act(free)), ("dve", c_dve(free))])
            if e == "act":
                if scale is None:
                    nc.scalar.activation(dst_ap, src_ap, Copy)
                else:
                    nc.scalar.activation(dst_ap, src_ap, Copy, scale=scale)
            else:
                if scale is None:
                    nc.vector.tensor_copy(dst_ap, src_ap)
                else:
                    nc.vector.tensor_scalar_mul(dst_ap, src_ap, scale)

        def emit_exp(pT_ap, pT_i8_ap, sT_ap):
            e = pick([("act", c_act(1024)), ("dve", c_dve(1024))])
            if e == "act":
                nc.scalar.activation(pT_ap, sT_ap, Exp,
                                     scale=float(SCALE) / QK_FACT)
            else:
                nc.vector.tensor_scalar(pT_i8_ap, sT_ap, A8, B8, MUL, ADD)

        # ---- prep: QKV projections (M=128: 4 heads per matmul) ----
        def emit_qk_tch(tch):
            for g in range(2):
                pt = sT_tile()
                for qk, w8 in enumerate((wq8, wk8)):
                    for a in range(2):
                        nc.tensor.matmul(
                            pt[:, qk, :],
                            w8[:, a, :, ds(128 * g, 128)],
                            xf[:, a, :, ts(tch, 512)],
                            start=(a == 0), stop=(a == 1),
                            perf_mode=DR,
                        )
                for qk, (bt, dT) in enumerate(((bq, qT), (bk, kT))):
                    for half in range(2):   # heads 4g+{0,1} then 4g+{2,3}
                        hp = 2 * g + half
                        if s_fp8:
                            dst = dT[0:64, hp, 0, ts(tch, 512)]
                        else:
                            dst = dT[0:64, hp, ts(tch, 512)]
                        emit_evict_qk(dst, pt[ds(64 * half, 64), qk, :],
                                      bt[ds(64 * half, 64), g:g + 1])

        def emit_v_tch(tch):
            for tt in range(4 * tch, 4 * tch + 4):
                vt = os_tile(tt % 2, "vps")
                for kc in range(4):
                    nc.tensor.matmul(
                        vt[:, 0:256],
                        x8[:, kc, ts(tt, 128)],
                        wv8[:, kc, :],
                        start=(kc == 0), stop=(kc == 3),
                    )
                dst = vx8[:, tt // 2, tt % 2, :, :, 0:32]
                e = pick([("act", c_act(256)), ("dve", c_dve(256))])
                src = vt[:, 0:256].rearrange("p (u m d) -> p u m d",
                                             u=2, m=4)
                if e == "act":
                    nc.scalar.activation(dst, src, Copy)
                else:
                    nc.vector.tensor_copy(dst, src)

        emit_qk_tch(0)
        emit_v_tch(0)
        emit_v_tch(1)
        for tch in range(1, QC):
            emit_qk_tch(tch)
        emit_v_tch(2)
        emit_v_tch(3)

        # ---- attention (head-serial) ----
        def emit_head(qc, h, os_ps, deferred=(), eager=False):
            hp, a = h // 2, h % 2
            dq = list(deferred)
            for ktp in range(KTP):
                if dq and (eager or ktp in (2, 5)):
                    dq.pop(0)()
                st = sT_tile()
                for i in range(2):
                    kt = 2 * ktp + i
                    if s_fp8:
                        nc.tensor.matmul(
                            st[:, i, :],
                            kT[ds(32 * a, 32), hp, :, ts(kt, 128)],
                            qT[ds(32 * a, 32), hp, :, ts(qc, 512)],
                            start=True, stop=True, perf_mode=DR,
                        )
                    else:
                        nc.tensor.matmul(
                            st[:, i, :],
                            kT[ds(32 * a, 32), hp, ts(kt, 128)],
                            qT[ds(32 * a, 32), hp, ts(qc, 512)],
                            start=True, stop=True,
                            tile_position=(32 * a, 0),
                        )
                pT = pTp.tile([128, 2, 512], f8, tag="pT", name="pT")
                emit_exp(pT[:], pT[:].bitcast(i8), st[:])
                u, m = h // 4, h % 4
                lhsT = vx8[:, ktp, :, u, :, :].rearrange(
                    "p i m c -> p i (m c)")[:, :, 128 * m:128 * m + 128]
                nc.tensor.matmul(
                    os_ps[0:128, :],
                    lhsT,
                    pT[:],
                    start=(ktp == 0), stop=(ktp == KTP - 1),
                    perf_mode=DR,
                )

        def emit_norm(qc, h, os_ps):
            c0, sp = c0_of(h), sp_of(h)
            r_sb = misc.tile([1, 512], f32, tag="r_sb", name="r_sb")
            nc.vector.reciprocal(r_sb[0:1, :], os_ps[ds(sp, 1), :])
            bal["dve"] += c_dve(512)
            rr = misc.tile([128, 512], f32, tag="rr", name="rr")
            nc.gpsimd.partition_broadcast(rr[:], r_sb[:])
            bal["pool"] += c_pool(512)
            dst = oT8[ds(c0, 32), h // 4, ts(qc, 512)]
            # path A: one DVE mul from psum; path B: ACT evict + Pool mul
            pool_mul = c_pool(512, eff=0.42)
            if bal["dve"] + c_dve(512) <= max(bal["act"] + c_act(512),
                                              bal["pool"] + pool_mul):
                e = "dve"
                bal["dve"] += c_dve(512)
            else:
                e = "actpool"
                bal["act"] += c_act(512)
            if e == "dve":
                nc.vector.tensor_mul(dst, os_ps[ds(c0, 32), :],
                                     rr[ds(c0, 32), :])
            else:
                o32 = misc.tile([128, 512], f32, tag="o32", name="o32")
                nc.scalar.activation(o32[ds(c0, 32), :],
                                     os_ps[ds(c0, 32), :], Copy)
                nc.gpsimd.tensor_mul(dst, o32[ds(c0, 32), :],
                                     rr[ds(c0, 32), :])
                bal["pool"] += c_pool(512, eff=0.42)

        def emit_proj_ct(qc, ct):
            pps = sT_tile()
            for yt in range(2):
                nc.tensor.matmul(
                    pps[:, 0, :], wp8[:, yt, ts(ct, 128)],
                    oT8[:, yt, ts(qc, 512)],
                    start=(yt == 0), stop=(yt == 1),
                )
            ost = ostage.tile([128, 512], f32, tag="ost", name="ost")
            emit_copy(ost[:], pps[:, 0, :], 512, scale=1.0 / W_SCALE)
            nc.sync.dma_start(
                outT_d.ap()[ts(ct, 128), ts(qc, 512)], ost[:])

        from collections import deque
        pending = deque()

        def make_norm(qc, h, os_ps):
            def f():
                emit_norm(qc, h, os_ps)
            return f

        def make_proj(qc, ct):
            def f():
                emit_proj_ct(qc, ct)
            return f

        for qc in range(QC):
            for h in range(HPC):
                os_ps = os_tile(h % 2, f"os{h % 2}")
                eager = qc == QC - 1 and h >= 4
                todo = []
                for _ in range(8 if eager else 2):
                    if pending:
                        todo.append(pending.popleft())
                emit_head(qc, h, os_ps, deferred=todo, eager=eager)
                pending.append(make_norm(qc, h, os_ps))
            # last head norm of this qc must land before its proj
            if qc < QC - 1:
                for ct in range(4):
                    pending.append(make_proj(qc, ct))
        while pending:
            pending.popleft()()
        for ct in range(4):
            emit_proj_ct(QC - 1, ct)

    nc.compile()
    nc._engine_balance = dict(bal)
    return nc


def _get_nc():
    if "nc" not in _CACHE:
        _CACHE["nc"] = _build()
    return _CACHE["nc"]


def kernel(x, w_attn, b_attn, w_proj, b_proj):
    from concourse.bass_utils import run_bass_kernel_spmd

    x = np.asarray(x, dtype=np.float32)
    w_attn = np.asarray(w_attn, dtype=np.float32)
    b_attn = np.asarray(b_attn, dtype=np.float32)
    w_proj = np.asarray(w_proj, dtype=np.float32)
    b_proj = np.asarray(b_proj, dtype=np.float32)

    nc = _get_nc()

    in_maps = []
    for core in range(N_CORES):
        b, hg = core // 2, core % 2
        cs = hg * 256
        # xb[p, kc, t] = x[b, t, 128*kc+p]; xf DR-packed fp8
        x8 = np.ascontiguousarray(
            x[b].T.reshape(4, 128, T).transpose(1, 0, 2)).astype(BF16)
        xf = np.ascontiguousarray(
            x[b].T.reshape(2, 2, 128, T).transpose(2, 0, 1, 3)).astype(F8)

        def wpack(w):  # [C, 256] -> [128, 4, 256], scaled, bf16
            return np.ascontiguousarray(
                (w * W_SCALE).reshape(4, 128, 256)
                .transpose(1, 0, 2)).astype(BF16)

        def wpack8(w):  # [C, 256] -> [128, 2, 2, 256], scaled, fp8
            return np.ascontiguousarray(
                (w * W_SCALE).reshape(2, 2, 128, 256)
                .transpose(2, 0, 1, 3)).astype(F8)

        wq8 = wpack8(w_attn[:, cs:cs + 256])
        wk8 = wpack8(w_attn[:, C + cs:C + cs + 256])
        wv8 = wpack(w_attn[:, 2 * C + cs:2 * C + cs + 256])
        bq = np.ascontiguousarray(
            (b_attn[cs:cs + 256] * W_SCALE).reshape(2, 128).T)
        bk = np.ascontiguousarray(
            (b_attn[C + cs:C + cs + 256] * W_SCALE).reshape(2, 128).T)
        # wp8[p, j, n] = w_proj[cs + 128j + p, n] * 64
        wp8 = np.ascontiguousarray(
            w_proj[cs:cs + 256, :].reshape(2, 128, C)
            .transpose(1, 0, 2)).astype(BF16)
        in_maps.append({
            "xb": x8, "xf": xf, "wqf": wq8, "wkf": wk8, "wvb": wv8,
            "bq": bq.astype(np.float32), "bk": bk.astype(np.float32),
            "wpb": wp8,
        })

    res = run_bass_kernel_spmd(nc, in_maps, core_ids=list(range(N_CORES)))

    b_eff = (b_proj + b_attn[2 * C:3 * C] @ w_proj).astype(np.float32)
    out = np.empty((B, T, C), dtype=np.float32)
    for b in range(B):
        acc = res.results[2 * b]["outT"].T + res.results[2 * b + 1]["outT"].T
        out[b] = acc + b_eff
    return out


if __name__ == "__main__":
    rng = np.random.default_rng(0)
    x = rng.standard_normal((B, T, C), dtype=np.float32)
    w_attn = rng.standard_normal((C, 3 * C), dtype=np.float32) * 0.02
    b_attn = rng.standard_normal(3 * C, dtype=np.float32) * 0.02
    w_proj = rng.standard_normal((C, C), dtype=np.float32) * 0.02
    b_proj = rng.standard_normal(C, dtype=np.float32) * 0.02
    out = kernel(x, w_attn, b_attn, w_proj, b_proj)
    print("kernel out", out.shape, out.dtype, float(np.abs(out).max()))
